# revision 5
# baseline (speedup 1.0000x reference)
"""CircuitGNN (3x TransformerConv + pool + MLP head) on 8 trn2 NeuronCores.

Strategy (graph/node-parallel with dst-side edge ownership):
  - Nodes are split into 8 contiguous ranges (padded to a multiple of 128).
  - Each layer: every core computes q/k/v/skip projections for its own
    nodes; k|v are packed into one [Np,512] table and AllGather'd so every
    core holds the full [8*Np,512] table; edges (sorted by destination,
    owned by the destination's core) gather k|v rows by source id via
    indirect DMA; attention + segment softmax + aggregation are computed
    with one-hot matrices on the tensor engine (segment-sum == matmul with
    a one-hot selection matrix).  exp() needs no max subtraction: logits
    are O(0.03) for this model family, and softmax is shift-invariant.
  - Mean-pool runs as a one-hot matmul accumulated over node blocks, an
    AllReduce combines the per-core partials, and the tiny global-MLP +
    regressor head runs replicated on every core.
"""

import math

import numpy as np

import concourse.bass as bass
import concourse.bacc as bacc
import concourse.mybir as mybir
import concourse.tile as tile
from concourse.bass import IndirectOffsetOnAxis
from concourse.masks import make_identity

P = 128
NCORES = 8
H = 4
HID = 64
DM = H * HID  # 256
FP = mybir.dt.float32
I32 = mybir.dt.int32
AF = mybir.ActivationFunctionType
ALU = mybir.AluOpType


# ---------------------------------------------------------------- host prep
def _prepare(x, global_features, params, edge_index, batch):
    N, D_IN = x.shape
    E = edge_index.shape[1]
    G = global_features.shape[0]

    src = np.asarray(edge_index[0], dtype=np.int64)
    dst = np.asarray(edge_index[1], dtype=np.int64)
    batch = np.asarray(batch, dtype=np.int64)

    n_per = math.ceil(N / NCORES)
    Np = math.ceil(n_per / P) * P
    NB = Np // P

    # global padded row id for a node
    core_of = src // n_per
    src_pad = core_of * Np + (src - core_of * n_per)

    order = np.argsort(dst, kind="stable")
    dst_s = dst[order]
    srcp_s = src_pad[order]

    # per (core, block) edge counts
    counts = np.zeros((NCORES, NB), dtype=np.int64)
    core_lims = np.searchsorted(dst_s, [c * n_per for c in range(NCORES + 1)])
    for c in range(NCORES):
        lo, hi = core_lims[c], core_lims[c + 1]
        dl = dst_s[lo:hi] - c * n_per
        blk_lims = np.searchsorted(dl, [b * P for b in range(NB + 1)])
        counts[c] = np.diff(blk_lims)

    K_b = np.maximum(1, np.ceil(counts.max(axis=0) / P).astype(np.int64))
    off = np.concatenate([[0], np.cumsum(K_b)])
    T_total = int(off[-1])

    src_idx = np.zeros((NCORES, T_total * P), dtype=np.int32)
    dst_loc = np.full((NCORES, T_total * P), -1.0, dtype=np.float32)
    for c in range(NCORES):
        lo, hi = core_lims[c], core_lims[c + 1]
        dl = dst_s[lo:hi] - c * n_per
        sp = srcp_s[lo:hi]
        blk_lims = np.searchsorted(dl, [b * P for b in range(NB + 1)])
        for b in range(NB):
            blo, bhi = blk_lims[b], blk_lims[b + 1]
            cnt = bhi - blo
            pos = off[b] * P + np.arange(cnt)
            src_idx[c, pos] = sp[blo:bhi]
            dst_loc[c, pos] = (dl[blo:bhi] - b * P).astype(np.float32)
    # [128, T_total] layout: col = tile, row = slot within tile
    src_idx = np.ascontiguousarray(
        src_idx.reshape(NCORES, T_total, P).transpose(0, 2, 1)
    )
    dst_loc = np.ascontiguousarray(
        dst_loc.reshape(NCORES, T_total, P).transpose(0, 2, 1)
    )

    # batch (graph id) per local node, [128, NB]; -1 on padded rows
    batchf = np.full((NCORES, Np), -1.0, dtype=np.float32)
    for c in range(NCORES):
        n_c = min(n_per, N - c * n_per)
        if n_c > 0:
            batchf[c, :n_c] = batch[c * n_per : c * n_per + n_c].astype(np.float32)
    batchf = np.ascontiguousarray(batchf.reshape(NCORES, NB, P).transpose(0, 2, 1))

    # node features, feature-major per core
    xT = np.zeros((NCORES, D_IN, Np), dtype=np.float32)
    for c in range(NCORES):
        n_c = min(n_per, N - c * n_per)
        if n_c > 0:
            xT[c, :, :n_c] = x[c * n_per : c * n_per + n_c].T

    f32 = lambda a: np.ascontiguousarray(np.asarray(a, dtype=np.float32))
    weights = {}
    dins = []
    for i in range(3):
        p = params[f"conv{i}"]
        din = p["Wq"].shape[1]
        dins.append(din)
        weights[f"wqk{i}"] = f32(np.concatenate([p["Wq"].T, p["Wk"].T], axis=1))
        weights[f"wvsk{i}"] = f32(np.concatenate([p["Wv"].T, p["Wskip"].T], axis=1))
        weights[f"bqk{i}"] = f32(np.concatenate([p["bq"], p["bk"]]).reshape(1, 2 * DM))
        weights[f"bvsk{i}"] = f32(
            np.concatenate([p["bv"], p["bskip"]]).reshape(1, 2 * DM)
        )
    for i, (W, b) in enumerate(params["gmlp"]):
        weights[f"wg{i}"] = f32(W.T)  # [K, GH]
        weights[f"bg{i}"] = f32(np.asarray(b).reshape(-1, 1))  # [GH, 1]
    (W0, b0), (W1, b1), (W2, b2) = params["reg"]
    weights["w0T"] = f32(W0.T)  # [DM+GH, RH]
    weights["b0"] = f32(np.asarray(b0).reshape(-1, 1))
    weights["w1T"] = f32(W1.T)
    weights["b1"] = f32(np.asarray(b1).reshape(-1, 1))
    weights["w2T"] = f32(W2.T)  # [RH, 1]
    weights["b2"] = f32(np.asarray(b2).reshape(1, 1))
    weights["gT"] = f32(np.asarray(global_features).T)  # [GIN, G]
    cnts = np.bincount(batch, minlength=G).astype(np.float32)
    weights["invcnt"] = f32((1.0 / np.maximum(cnts, 1.0)).reshape(1, G))

    meta = dict(
        N=N, E=E, G=G, D_IN=D_IN, GH=params["gmlp"][0][0].shape[0],
        RH=W0.shape[0], GIN=global_features.shape[1],
        n_per=n_per, Np=Np, NB=NB, K_b=[int(k) for k in K_b],
        off=[int(o) for o in off], T_total=T_total, dins=dins,
    )
    percore = dict(src_idx=src_idx, dst_loc=dst_loc, batchf=batchf, xT=xT)
    return meta, percore, weights


# ---------------------------------------------------------------- program
def _build(meta):
    Np, NB = meta["Np"], meta["NB"]
    K_b, off, T_total = meta["K_b"], meta["off"], meta["T_total"]
    G, GH, RH, GIN, D_IN = meta["G"], meta["GH"], meta["RH"], meta["GIN"], meta["D_IN"]
    dins = meta["dins"]

    nc = bacc.Bacc(
        "TRN2", target_bir_lowering=False, debug=False, num_devices=NCORES
    )
    rg = [list(range(NCORES))]

    # -------- I/O
    xT_in = nc.dram_tensor("xT", [D_IN, Np], FP, kind="ExternalInput")
    srci_in = nc.dram_tensor("src_idx", [P, T_total], I32, kind="ExternalInput")
    dstl_in = nc.dram_tensor("dst_loc", [P, T_total], FP, kind="ExternalInput")
    batch_in = nc.dram_tensor("batchf", [P, NB], FP, kind="ExternalInput")
    win = {}
    for i in range(3):
        din = dins[i]
        win[f"wqk{i}"] = nc.dram_tensor(f"wqk{i}", [din, 2 * DM], FP, kind="ExternalInput")
        win[f"wvsk{i}"] = nc.dram_tensor(f"wvsk{i}", [din, 2 * DM], FP, kind="ExternalInput")
        win[f"bqk{i}"] = nc.dram_tensor(f"bqk{i}", [1, 2 * DM], FP, kind="ExternalInput")
        win[f"bvsk{i}"] = nc.dram_tensor(f"bvsk{i}", [1, 2 * DM], FP, kind="ExternalInput")
    gk = [GIN, GH, GH]
    for i in range(3):
        win[f"wg{i}"] = nc.dram_tensor(f"wg{i}", [gk[i], GH], FP, kind="ExternalInput")
        win[f"bg{i}"] = nc.dram_tensor(f"bg{i}", [GH, 1], FP, kind="ExternalInput")
    win["w0T"] = nc.dram_tensor("w0T", [DM + GH, RH], FP, kind="ExternalInput")
    win["b0"] = nc.dram_tensor("b0", [RH, 1], FP, kind="ExternalInput")
    win["w1T"] = nc.dram_tensor("w1T", [RH, RH], FP, kind="ExternalInput")
    win["b1"] = nc.dram_tensor("b1", [RH, 1], FP, kind="ExternalInput")
    win["w2T"] = nc.dram_tensor("w2T", [RH, 1], FP, kind="ExternalInput")
    win["b2"] = nc.dram_tensor("b2", [1, 1], FP, kind="ExternalInput")
    win["gT"] = nc.dram_tensor("gT", [GIN, G], FP, kind="ExternalInput")
    win["invcnt"] = nc.dram_tensor("invcnt", [1, G], FP, kind="ExternalInput")
    out_dram = nc.dram_tensor("out", [1, G], FP, kind="ExternalOutput")

    with tile.TileContext(nc) as tc:
        with (
            tc.tile_pool(name="const", bufs=1) as cpool,
            tc.tile_pool(name="dram", bufs=1, space="DRAM") as dpool,
            tc.tile_pool(name="work", bufs=3) as wpool,
            tc.tile_pool(name="io", bufs=2) as iopool,
            tc.tile_pool(name="psum", bufs=2, space="PSUM") as ppool,
        ):
            # ---- persistent DRAM scratch
            hT_a = dpool.tile([DM, Np], FP, name="hT_a")
            hT_b = dpool.tile([DM, Np], FP, name="hT_b")
            q_dram = dpool.tile([Np, DM], FP, name="q_dram")
            skip_dram = dpool.tile([Np, DM], FP, name="skip_dram")
            kv_agin = [
                dpool.tile([Np, 2 * DM], FP, name=f"kv_agin{i}") for i in range(3)
            ]
            kv_full = [
                dpool.tile(
                    [NCORES * Np, 2 * DM], FP, name=f"kv_full{i}", addr_space="Shared"
                )
                for i in range(3)
            ]
            pool_in = dpool.tile([DM, G], FP, name="pool_in")
            pool_red = dpool.tile([DM, G], FP, name="pool_red", addr_space="Shared")

            # ---- constants / persistent SBUF
            identity = cpool.tile([P, P], FP, name="identity")
            make_identity(nc, identity[:])
            iota_f = cpool.tile([P, P], FP, name="iota_f")
            nc.gpsimd.iota(
                iota_f[:], [[1, P]], channel_multiplier=0,
                allow_small_or_imprecise_dtypes=True,
            )
            ones_row = cpool.tile([1, P], FP, name="ones_row")
            nc.gpsimd.memset(ones_row[:], 1.0)

            srci_s = cpool.tile([P, T_total], I32, name="srci_s")
            nc.sync.dma_start(out=srci_s[:], in_=srci_in[:, :])
            dstl_s = cpool.tile([P, T_total], FP, name="dstl_s")
            nc.sync.dma_start(out=dstl_s[:], in_=dstl_in[:, :])
            batch_s = cpool.tile([P, NB], FP, name="batch_s")
            nc.sync.dma_start(out=batch_s[:], in_=batch_in[:, :])

            wsb = {}
            for i in range(3):
                din = dins[i]
                nk = math.ceil(din / P)
                for nm in (f"wqk{i}", f"wvsk{i}"):
                    for kc in range(nk):
                        klo, khi = kc * P, min((kc + 1) * P, din)
                        t = cpool.tile([khi - klo, 2 * DM], FP, name=f"{nm}_{kc}")
                        nc.sync.dma_start(out=t[:], in_=win[nm][klo:khi, :])
                        wsb[f"{nm}_{kc}"] = t
                for nm in (f"bqk{i}", f"bvsk{i}"):
                    t = cpool.tile([1, 2 * DM], FP, name=f"{nm}_s")
                    nc.sync.dma_start(out=t[:], in_=win[nm][:, :])
                    wsb[nm] = t
            for nm, shp in [
                ("wg0", [gk[0], GH]), ("wg1", [gk[1], GH]), ("wg2", [gk[2], GH]),
                ("bg0", [GH, 1]), ("bg1", [GH, 1]), ("bg2", [GH, 1]),
                ("b0", [RH, 1]), ("b1", [RH, 1]), ("w2T", [RH, 1]), ("b2", [1, 1]),
                ("w1T", [RH, RH]), ("gT", [GIN, G]),
            ]:
                t = cpool.tile(shp, FP, name=f"{nm}_s")
                nc.sync.dma_start(out=t[:], in_=win[nm][:, :])
                wsb[nm] = t
            for kc in range(3):  # w0T chunks: [128,RH],[128,RH],[GH,RH]
                klo = kc * P
                khi = min(klo + P, DM + GH)
                t = cpool.tile([khi - klo, RH], FP, name=f"w0T_{kc}")
                nc.sync.dma_start(out=t[:], in_=win["w0T"][klo:khi, :])
                wsb[f"w0T_{kc}"] = t
            inv1 = cpool.tile([1, G], FP, name="inv1")
            nc.sync.dma_start(out=inv1[:], in_=win["invcnt"][:, :])
            invb = cpool.tile([P, G], FP, name="invb")
            nc.gpsimd.partition_broadcast(invb[:], inv1[:])

            pool_acc0 = cpool.tile([P, G], FP, name="pool_acc0")
            pool_acc1 = cpool.tile([P, G], FP, name="pool_acc1")
            nc.vector.memset(pool_acc0[:], 0.0)
            nc.vector.memset(pool_acc1[:], 0.0)

            # ---------------- per-layer passes
            def pass1(layer, h_src):
                din = dins[layer]
                nk = math.ceil(din / P)
                for nt in range(NB):
                    cols = slice(nt * P, (nt + 1) * P)
                    hT_t = []
                    for kc in range(nk):
                        klo, khi = kc * P, min((kc + 1) * P, din)
                        t = wpool.tile([khi - klo, P], FP, name="hT_t", tag="hT_t")
                        nc.sync.dma_start(out=t[:], in_=h_src[klo:khi, cols])
                        hT_t.append(t)
                    outs = []
                    for half, wn, bn in (
                        (0, f"wqk{layer}", f"bqk{layer}"),
                        (1, f"wvsk{layer}", f"bvsk{layer}"),
                    ):
                        ps = ppool.tile([P, 2 * DM], FP, name="p1", tag="p1")
                        for kc in range(nk):
                            nc.tensor.matmul(
                                ps[:], lhsT=hT_t[kc][:], rhs=wsb[f"{wn}_{kc}"][:],
                                start=(kc == 0), stop=False,
                            )
                        nc.tensor.matmul(
                            ps[:], lhsT=ones_row[:], rhs=wsb[bn][:],
                            start=False, stop=True,
                        )
                        sb = wpool.tile([P, 2 * DM], FP, name="p1sb", tag="p1sb")
                        if half == 0:
                            nc.vector.tensor_copy(sb[:], ps[:])
                        else:
                            nc.scalar.copy(sb[:], ps[:])
                        outs.append(sb)
                    qk, vsk = outs
                    rows = slice(nt * P, (nt + 1) * P)
                    nc.sync.dma_start(out=q_dram[rows, :], in_=qk[:, 0:DM])
                    nc.sync.dma_start(
                        out=kv_agin[layer][rows, 0:DM], in_=qk[:, DM : 2 * DM]
                    )
                    nc.sync.dma_start(
                        out=kv_agin[layer][rows, DM : 2 * DM], in_=vsk[:, 0:DM]
                    )
                    nc.sync.dma_start(out=skip_dram[rows, :], in_=vsk[:, DM : 2 * DM])

            def pass2(layer, hT_next):
                for b in range(NB):
                    rows = slice(b * P, (b + 1) * P)
                    q_t = iopool.tile([P, DM], FP, name="q_t", tag="q_t")
                    nc.sync.dma_start(out=q_t[:], in_=q_dram[rows, :])
                    sk_t = iopool.tile([P, DM], FP, name="sk_t", tag="sk_t")
                    nc.sync.dma_start(out=sk_t[:], in_=skip_dram[rows, :])
                    seg = ppool.tile([P, 4 + DM], FP, name="seg", tag="seg")
                    for t in range(K_b[b]):
                        gt = off[b] + t
                        kv_g = wpool.tile([P, 2 * DM], FP, name="kv_g", tag="kv_g")
                        nc.gpsimd.indirect_dma_start(
                            out=kv_g[:],
                            out_offset=None,
                            in_=kv_full[layer][:, :],
                            in_offset=IndirectOffsetOnAxis(
                                ap=srci_s[:, gt : gt + 1], axis=0
                            ),
                        )
                        P_t = wpool.tile([P, P], FP, name="P_t", tag="P_t")
                        nc.vector.tensor_tensor(
                            out=P_t[:],
                            in0=dstl_s[:, gt : gt + 1].to_broadcast([P, P]),
                            in1=iota_f[:],
                            op=ALU.is_equal,
                        )
                        ptp = ppool.tile([P, P], FP, name="ptp", tag="ptp")
                        nc.tensor.transpose(ptp[:], P_t[:], identity[:])
                        PT_s = wpool.tile([P, P], FP, name="PT_s", tag="PT_s")
                        nc.scalar.copy(PT_s[:], ptp[:])
                        qe = ppool.tile([P, DM], FP, name="qe", tag="qe")
                        nc.tensor.matmul(
                            qe[:], lhsT=PT_s[:], rhs=q_t[:], start=True, stop=True
                        )
                        payload = wpool.tile([P, 4 + DM], FP, name="payload", tag="payload")
                        alpha4 = wpool.tile([P, 4], FP, name="alpha4", tag="alpha4")
                        prod = wpool.tile([P, DM], FP, name="prod", tag="prod")
                        nc.vector.tensor_mul(prod[:], qe[:], kv_g[:, 0:DM])
                        nc.vector.tensor_reduce(
                            out=alpha4[:],
                            in_=prod[:].rearrange("p (h c) -> p h c", c=HID),
                            axis=mybir.AxisListType.X,
                            op=ALU.add,
                        )
                        nc.scalar.activation(
                            payload[:, 0:4], alpha4[:], AF.Exp,
                            scale=1.0 / math.sqrt(HID),
                        )
                        for h in range(H):
                            nc.vector.tensor_scalar(
                                out=payload[:, 4 + h * HID : 4 + (h + 1) * HID],
                                in0=kv_g[:, DM + h * HID : DM + (h + 1) * HID],
                                scalar1=payload[:, h : h + 1],
                                scalar2=None,
                                op0=ALU.mult,
                            )
                        nc.tensor.matmul(
                            seg[:], lhsT=P_t[:], rhs=payload[:],
                            start=(t == 0), stop=(t == K_b[b] - 1),
                        )
                    # epilogue
                    rec4 = wpool.tile([P, 4], FP, name="rec4", tag="rec4")
                    nc.vector.tensor_scalar(
                        out=rec4[:], in0=seg[:, 0:4], scalar1=1e-20, scalar2=None,
                        op0=ALU.max,
                    )
                    nc.vector.reciprocal(rec4[:], rec4[:])
                    hb = wpool.tile([P, DM], FP, name="hb", tag="hb")
                    for h in range(H):
                        nc.vector.tensor_scalar(
                            out=hb[:, h * HID : (h + 1) * HID],
                            in0=seg[:, 4 + h * HID : 4 + (h + 1) * HID],
                            scalar1=rec4[:, h : h + 1],
                            scalar2=None,
                            op0=ALU.mult,
                        )
                    nc.vector.tensor_add(hb[:], hb[:], sk_t[:])
                    hr = wpool.tile([P, DM], FP, name="hr", tag="hr")
                    nc.scalar.activation(hr[:], hb[:], AF.Relu)
                    if hT_next is not None:
                        for half in range(2):
                            tp = ppool.tile([P, P], FP, name="ptp", tag="ptp")
                            nc.tensor.transpose(
                                tp[:], hr[:, half * P : (half + 1) * P], identity[:]
                            )
                            ts = wpool.tile([P, P], FP, name="ts", tag="ts")
                            nc.vector.tensor_copy(ts[:], tp[:])
                            nc.sync.dma_start(
                                out=hT_next[half * P : (half + 1) * P, rows], in_=ts[:]
                            )
                    else:
                        B_t = wpool.tile([P, G], FP, name="B_t", tag="B_t")
                        nc.vector.tensor_tensor(
                            out=B_t[:],
                            in0=batch_s[:, b : b + 1].to_broadcast([P, G]),
                            in1=iota_f[:, 0:G],
                            op=ALU.is_equal,
                        )
                        for half, acc in ((0, pool_acc0), (1, pool_acc1)):
                            pp = ppool.tile([P, G], FP, name="ptp", tag="ptp")
                            nc.tensor.matmul(
                                pp[:], lhsT=hr[:, half * P : (half + 1) * P],
                                rhs=B_t[:], start=True, stop=True,
                            )
                            nc.vector.tensor_add(acc[:], acc[:], pp[:])

            def ag_kv(layer):
                nc.gpsimd.collective_compute(
                    "AllGather",
                    ALU.bypass,
                    replica_groups=rg,
                    ins=[kv_agin[layer].opt()],
                    outs=[kv_full[layer].opt()],
                )

            pass1(0, xT_in)
            ag_kv(0)
            pass2(0, hT_a)
            pass1(1, hT_a)
            ag_kv(1)
            pass2(1, hT_b)
            pass1(2, hT_b)
            ag_kv(2)
            pass2(2, None)

            # ---------------- head
            nc.sync.dma_start(out=pool_in[0:P, :], in_=pool_acc0[:])
            nc.sync.dma_start(out=pool_in[P : 2 * P, :], in_=pool_acc1[:])
            nc.gpsimd.collective_compute(
                "AllReduce",
                ALU.add,
                replica_groups=rg,
                ins=[pool_in.opt()],
                outs=[pool_red.opt()],
            )
            xp = []
            for half in range(2):
                t = iopool.tile([P, G], FP, name="poolr", tag="poolr")
                nc.sync.dma_start(out=t[:], in_=pool_red[half * P : (half + 1) * P, :])
                m = cpool.tile([P, G], FP, name=f"xp{half}")
                nc.vector.tensor_mul(m[:], t[:], invb[:])
                xp.append(m)
            # global mlp branch (feature-major [GH, G])
            g_cur = wsb["gT"]
            for i in range(3):
                gp = ppool.tile([GH, G], FP, name="p1", tag="p1")
                nc.tensor.matmul(
                    gp[:], lhsT=wsb[f"wg{i}"][:], rhs=g_cur[:], start=True, stop=True
                )
                gs = cpool.tile([GH, G], FP, name=f"g_{i}")
                nc.scalar.activation(
                    gs[:], gp[:], AF.Relu, bias=wsb[f"bg{i}"][:, 0:1]
                )
                g_cur = gs
            # regressor
            r0p = ppool.tile([RH, G], FP, name="p1", tag="p1")
            nc.tensor.matmul(r0p[:], lhsT=wsb["w0T_0"][:], rhs=xp[0][:], start=True, stop=False)
            nc.tensor.matmul(r0p[:], lhsT=wsb["w0T_1"][:], rhs=xp[1][:], start=False, stop=False)
            nc.tensor.matmul(r0p[:], lhsT=wsb["w0T_2"][:], rhs=g_cur[:], start=False, stop=True)
            r0 = cpool.tile([RH, G], FP, name="r0")
            nc.scalar.activation(r0[:], r0p[:], AF.Relu, bias=wsb["b0"][:, 0:1])
            r1p = ppool.tile([RH, G], FP, name="p1", tag="p1")
            nc.tensor.matmul(r1p[:], lhsT=wsb["w1T"][:], rhs=r0[:], start=True, stop=True)
            r1 = cpool.tile([RH, G], FP, name="r1")
            nc.scalar.activation(r1[:], r1p[:], AF.Relu, bias=wsb["b1"][:, 0:1])
            r2p = ppool.tile([1, G], FP, name="p1", tag="p1")
            nc.tensor.matmul(r2p[:], lhsT=wsb["w2T"][:], rhs=r1[:], start=True, stop=True)
            outs = cpool.tile([1, G], FP, name="outs")
            nc.scalar.activation(outs[:], r2p[:], AF.Identity, bias=wsb["b2"][:, 0:1])
            nc.sync.dma_start(out=out_dram[:, :], in_=outs[:])

    nc.compile()
    return nc


def _in_maps(meta, percore, weights):
    maps = []
    for c in range(NCORES):
        m = dict(
            xT=percore["xT"][c],
            src_idx=percore["src_idx"][c],
            dst_loc=percore["dst_loc"][c],
            batchf=percore["batchf"][c],
        )
        m.update(weights)
        maps.append(m)
    return maps


LAST_RESULT = None


def kernel(x, global_features, params, edge_index, batch):
    global LAST_RESULT
    from concourse.bass_utils import run_bass_kernel_spmd

    meta, percore, weights = _prepare(x, global_features, params, edge_index, batch)
    nc = _build(meta)
    maps = _in_maps(meta, percore, weights)
    res = run_bass_kernel_spmd(nc, maps, core_ids=list(range(NCORES)))
    LAST_RESULT = res
    return np.asarray(res.results[0]["out"], dtype=np.float32).reshape(-1)


# revision 9
# speedup vs baseline: 1.4503x; 1.4503x over previous
"""CircuitGNN (3x TransformerConv + pool + MLP head) on 8 trn2 NeuronCores.

Strategy (graph/node-parallel with dst-side edge ownership):
  - Nodes are split into 8 contiguous ranges (padded to a multiple of 128).
  - Each layer: every core computes q/k/v/skip projections for its own
    nodes; k|v are packed into one [Np,512] table and AllGather'd so every
    core holds the full [8*Np,512] table; edges (sorted by destination,
    owned by the destination's core) gather k|v rows by source id via
    indirect DMA; attention + segment softmax + aggregation are computed
    with one-hot matrices on the tensor engine (segment-sum == matmul with
    a one-hot selection matrix).  exp() needs no max subtraction: logits
    are O(0.03) for this model family, and softmax is shift-invariant.
  - Mean-pool runs as a one-hot matmul accumulated over node blocks, an
    AllReduce combines the per-core partials, and the tiny global-MLP +
    regressor head runs replicated on every core.
"""

import math

import numpy as np

import concourse.bass as bass
import concourse.bacc as bacc
import concourse.mybir as mybir
import concourse.tile as tile
from concourse.bass import IndirectOffsetOnAxis
from concourse.masks import make_identity

BF16_NP = mybir.dt.np(mybir.dt.bfloat16)

P = 128
NCORES = 8
H = 4
HID = 64
DM = H * HID  # 256
FP = mybir.dt.float32
BF = mybir.dt.bfloat16
I32 = mybir.dt.int32
AF = mybir.ActivationFunctionType
ALU = mybir.AluOpType


# ---------------------------------------------------------------- host prep
def _prepare(x, global_features, params, edge_index, batch):
    N, D_IN = x.shape
    E = edge_index.shape[1]
    G = global_features.shape[0]

    src = np.asarray(edge_index[0], dtype=np.int64)
    dst = np.asarray(edge_index[1], dtype=np.int64)
    batch = np.asarray(batch, dtype=np.int64)

    n_per = math.ceil(N / NCORES)
    Np = math.ceil(n_per / P) * P
    NB = Np // P

    # global padded row id for a node
    core_of = src // n_per
    src_pad = core_of * Np + (src - core_of * n_per)

    order = np.argsort(dst, kind="stable")
    dst_s = dst[order]
    srcp_s = src_pad[order]

    # per (core, block) edge counts
    counts = np.zeros((NCORES, NB), dtype=np.int64)
    core_lims = np.searchsorted(dst_s, [c * n_per for c in range(NCORES + 1)])
    for c in range(NCORES):
        lo, hi = core_lims[c], core_lims[c + 1]
        dl = dst_s[lo:hi] - c * n_per
        blk_lims = np.searchsorted(dl, [b * P for b in range(NB + 1)])
        counts[c] = np.diff(blk_lims)

    K_b = np.maximum(1, np.ceil(counts.max(axis=0) / P).astype(np.int64))
    off = np.concatenate([[0], np.cumsum(K_b)])
    T_total = int(off[-1])

    src_idx = np.zeros((NCORES, T_total * P), dtype=np.int32)
    dst_loc = np.full((NCORES, T_total * P), -1.0, dtype=np.float32)
    for c in range(NCORES):
        lo, hi = core_lims[c], core_lims[c + 1]
        dl = dst_s[lo:hi] - c * n_per
        sp = srcp_s[lo:hi]
        blk_lims = np.searchsorted(dl, [b * P for b in range(NB + 1)])
        for b in range(NB):
            blo, bhi = blk_lims[b], blk_lims[b + 1]
            cnt = bhi - blo
            pos = off[b] * P + np.arange(cnt)
            src_idx[c, pos] = sp[blo:bhi]
            dst_loc[c, pos] = (dl[blo:bhi] - b * P).astype(np.float32)
    # [128, T_total] layout: col = tile, row = slot within tile
    src_idx = np.ascontiguousarray(
        src_idx.reshape(NCORES, T_total, P).transpose(0, 2, 1)
    )
    dst_loc = np.ascontiguousarray(
        dst_loc.reshape(NCORES, T_total, P).transpose(0, 2, 1)
    )

    # batch (graph id) per local node, [128, NB]; -1 on padded rows
    batchf = np.full((NCORES, Np), -1.0, dtype=np.float32)
    for c in range(NCORES):
        n_c = min(n_per, N - c * n_per)
        if n_c > 0:
            batchf[c, :n_c] = batch[c * n_per : c * n_per + n_c].astype(np.float32)
    batchf = np.ascontiguousarray(batchf.reshape(NCORES, NB, P).transpose(0, 2, 1))

    # node features, feature-major per core
    xT = np.zeros((NCORES, D_IN, Np), dtype=BF16_NP)
    for c in range(NCORES):
        n_c = min(n_per, N - c * n_per)
        if n_c > 0:
            xT[c, :, :n_c] = x[c * n_per : c * n_per + n_c].T.astype(BF16_NP)

    f32 = lambda a: np.ascontiguousarray(np.asarray(a, dtype=np.float32))
    bf16 = lambda a: np.ascontiguousarray(np.asarray(a, dtype=np.float32).astype(BF16_NP))
    weights = {}
    dins = []
    for i in range(3):
        p = params[f"conv{i}"]
        din = p["Wq"].shape[1]
        dins.append(din)
        weights[f"wqk{i}"] = bf16(np.concatenate([p["Wq"].T, p["Wk"].T], axis=1))
        weights[f"wvsk{i}"] = bf16(np.concatenate([p["Wv"].T, p["Wskip"].T], axis=1))
        weights[f"bqk{i}"] = bf16(np.concatenate([p["bq"], p["bk"]]).reshape(1, 2 * DM))
        weights[f"bvsk{i}"] = bf16(
            np.concatenate([p["bv"], p["bskip"]]).reshape(1, 2 * DM)
        )
    for i, (W, b) in enumerate(params["gmlp"]):
        weights[f"wg{i}"] = f32(W.T)  # [K, GH]
        weights[f"bg{i}"] = f32(np.asarray(b).reshape(-1, 1))  # [GH, 1]
    (W0, b0), (W1, b1), (W2, b2) = params["reg"]
    weights["w0T"] = f32(W0.T)  # [DM+GH, RH]
    weights["b0"] = f32(np.asarray(b0).reshape(-1, 1))
    weights["w1T"] = f32(W1.T)
    weights["b1"] = f32(np.asarray(b1).reshape(-1, 1))
    weights["w2T"] = f32(W2.T)  # [RH, 1]
    weights["b2"] = f32(np.asarray(b2).reshape(1, 1))
    weights["gT"] = f32(np.asarray(global_features).T)  # [GIN, G]
    cnts = np.bincount(batch, minlength=G).astype(np.float32)
    weights["invcnt"] = f32((1.0 / np.maximum(cnts, 1.0)).reshape(1, G))

    meta = dict(
        N=N, E=E, G=G, D_IN=D_IN, GH=params["gmlp"][0][0].shape[0],
        RH=W0.shape[0], GIN=global_features.shape[1],
        n_per=n_per, Np=Np, NB=NB, K_b=[int(k) for k in K_b],
        off=[int(o) for o in off], T_total=T_total, dins=dins,
    )
    percore = dict(src_idx=src_idx, dst_loc=dst_loc, batchf=batchf, xT=xT)
    return meta, percore, weights


# ---------------------------------------------------------------- program
USE_TDMA = False  # transpose-DMA loads for h (vs PE-transpose in epilogue)


def _build(meta):
    Np, NB = meta["Np"], meta["NB"]
    K_b, off, T_total = meta["K_b"], meta["off"], meta["T_total"]
    G, GH, RH, GIN, D_IN = meta["G"], meta["GH"], meta["RH"], meta["GIN"], meta["D_IN"]
    dins = meta["dins"]

    nc = bacc.Bacc(
        "TRN2", target_bir_lowering=False, debug=False, num_devices=NCORES
    )
    rg = [list(range(NCORES))]

    # -------- I/O
    xT_in = nc.dram_tensor("xT", [D_IN, Np], BF, kind="ExternalInput")
    srci_in = nc.dram_tensor("src_idx", [P, T_total], I32, kind="ExternalInput")
    dstl_in = nc.dram_tensor("dst_loc", [P, T_total], FP, kind="ExternalInput")
    batch_in = nc.dram_tensor("batchf", [P, NB], FP, kind="ExternalInput")
    win = {}
    for i in range(3):
        din = dins[i]
        win[f"wqk{i}"] = nc.dram_tensor(f"wqk{i}", [din, 2 * DM], BF, kind="ExternalInput")
        win[f"wvsk{i}"] = nc.dram_tensor(f"wvsk{i}", [din, 2 * DM], BF, kind="ExternalInput")
        win[f"bqk{i}"] = nc.dram_tensor(f"bqk{i}", [1, 2 * DM], BF, kind="ExternalInput")
        win[f"bvsk{i}"] = nc.dram_tensor(f"bvsk{i}", [1, 2 * DM], BF, kind="ExternalInput")
    gk = [GIN, GH, GH]
    for i in range(3):
        win[f"wg{i}"] = nc.dram_tensor(f"wg{i}", [gk[i], GH], FP, kind="ExternalInput")
        win[f"bg{i}"] = nc.dram_tensor(f"bg{i}", [GH, 1], FP, kind="ExternalInput")
    win["w0T"] = nc.dram_tensor("w0T", [DM + GH, RH], FP, kind="ExternalInput")
    win["b0"] = nc.dram_tensor("b0", [RH, 1], FP, kind="ExternalInput")
    win["w1T"] = nc.dram_tensor("w1T", [RH, RH], FP, kind="ExternalInput")
    win["b1"] = nc.dram_tensor("b1", [RH, 1], FP, kind="ExternalInput")
    win["w2T"] = nc.dram_tensor("w2T", [RH, 1], FP, kind="ExternalInput")
    win["b2"] = nc.dram_tensor("b2", [1, 1], FP, kind="ExternalInput")
    win["gT"] = nc.dram_tensor("gT", [GIN, G], FP, kind="ExternalInput")
    win["invcnt"] = nc.dram_tensor("invcnt", [1, G], FP, kind="ExternalInput")
    out_dram = nc.dram_tensor("out", [1, G], FP, kind="ExternalOutput")

    with tile.TileContext(nc) as tc:
        with (
            tc.tile_pool(name="const", bufs=1) as cpool,
            tc.tile_pool(name="dram", bufs=1, space="DRAM") as dpool,
            tc.tile_pool(name="work", bufs=3) as wpool,
            tc.tile_pool(name="io", bufs=2) as iopool,
            tc.tile_pool(name="psum", bufs=2, space="PSUM") as ppool,
        ):
            # ---- persistent DRAM scratch
            if USE_TDMA:
                h_a = dpool.tile([Np, DM], BF, name="h_a")
                h_b = dpool.tile([Np, DM], BF, name="h_b")
            else:
                h_a = dpool.tile([DM, Np], BF, name="h_a")
                h_b = dpool.tile([DM, Np], BF, name="h_b")
            q_dram = dpool.tile([Np, DM], BF, name="q_dram")
            skip_dram = dpool.tile([Np, DM], BF, name="skip_dram")
            kv_agin = [
                dpool.tile([Np, 2 * DM], BF, name=f"kv_agin{i}") for i in range(3)
            ]
            kv_full = [
                dpool.tile(
                    [NCORES * Np, 2 * DM], BF, name=f"kv_full{i}", addr_space="Shared"
                )
                for i in range(3)
            ]
            pool_in = dpool.tile([DM, G], FP, name="pool_in")
            pool_red = dpool.tile([DM, G], FP, name="pool_red", addr_space="Shared")

            # ---- constants / persistent SBUF
            identity = cpool.tile([P, P], BF, name="identity")
            make_identity(nc, identity[:])
            iota_f = cpool.tile([P, P], FP, name="iota_f")
            nc.gpsimd.iota(
                iota_f[:], [[1, P]], channel_multiplier=0,
                allow_small_or_imprecise_dtypes=True,
            )
            ones_row = cpool.tile([1, P], BF, name="ones_row")
            nc.gpsimd.memset(ones_row[:], 1.0)

            srci_s = cpool.tile([P, T_total], I32, name="srci_s")
            nc.sync.dma_start(out=srci_s[:], in_=srci_in[:, :])
            dstl_s = cpool.tile([P, T_total], FP, name="dstl_s")
            nc.sync.dma_start(out=dstl_s[:], in_=dstl_in[:, :])
            batch_s = cpool.tile([P, NB], FP, name="batch_s")
            nc.sync.dma_start(out=batch_s[:], in_=batch_in[:, :])

            wsb = {}
            for i in range(3):
                din = dins[i]
                nk = math.ceil(din / P)
                for nm in (f"wqk{i}", f"wvsk{i}"):
                    for kc in range(nk):
                        klo, khi = kc * P, min((kc + 1) * P, din)
                        t = cpool.tile([khi - klo, 2 * DM], BF, name=f"{nm}_{kc}")
                        nc.sync.dma_start(out=t[:], in_=win[nm][klo:khi, :])
                        wsb[f"{nm}_{kc}"] = t
                for nm in (f"bqk{i}", f"bvsk{i}"):
                    t = cpool.tile([1, 2 * DM], BF, name=f"{nm}_s")
                    nc.sync.dma_start(out=t[:], in_=win[nm][:, :])
                    wsb[nm] = t
            for nm, shp in [
                ("wg0", [gk[0], GH]), ("wg1", [gk[1], GH]), ("wg2", [gk[2], GH]),
                ("bg0", [GH, 1]), ("bg1", [GH, 1]), ("bg2", [GH, 1]),
                ("b0", [RH, 1]), ("b1", [RH, 1]), ("w2T", [RH, 1]), ("b2", [1, 1]),
                ("w1T", [RH, RH]), ("gT", [GIN, G]),
            ]:
                t = cpool.tile(shp, FP, name=f"{nm}_s")
                nc.sync.dma_start(out=t[:], in_=win[nm][:, :])
                wsb[nm] = t
            for kc in range(3):  # w0T chunks: [128,RH],[128,RH],[GH,RH]
                klo = kc * P
                khi = min(klo + P, DM + GH)
                t = cpool.tile([khi - klo, RH], FP, name=f"w0T_{kc}")
                nc.sync.dma_start(out=t[:], in_=win["w0T"][klo:khi, :])
                wsb[f"w0T_{kc}"] = t
            inv1 = cpool.tile([1, G], FP, name="inv1")
            nc.sync.dma_start(out=inv1[:], in_=win["invcnt"][:, :])
            invb = cpool.tile([P, G], FP, name="invb")
            nc.gpsimd.partition_broadcast(invb[:], inv1[:])

            pool_acc0 = cpool.tile([P, G], FP, name="pool_acc0")
            pool_acc1 = cpool.tile([P, G], FP, name="pool_acc1")
            nc.vector.memset(pool_acc0[:], 0.0)
            nc.vector.memset(pool_acc1[:], 0.0)

            # ---------------- per-layer passes
            def pass1(layer, h_src):
                din = dins[layer]
                nk = math.ceil(din / P)
                for nt in range(NB):
                    rows = slice(nt * P, (nt + 1) * P)
                    hT_t = []
                    for kc in range(nk):
                        klo, khi = kc * P, min((kc + 1) * P, din)
                        t = wpool.tile([khi - klo, P], BF, name="hT_t", tag="hT_t")
                        if layer == 0 or not USE_TDMA:
                            nc.sync.dma_start(out=t[:], in_=h_src[klo:khi, rows])
                        else:
                            nc.sync.dma_start(
                                out=t[:], in_=h_src[rows, klo:khi], transpose=True
                            )
                        hT_t.append(t)
                    outs = []
                    for half, wn, bn in (
                        (0, f"wqk{layer}", f"bqk{layer}"),
                        (1, f"wvsk{layer}", f"bvsk{layer}"),
                    ):
                        ps = ppool.tile([P, 2 * DM], FP, name="p1", tag="p1")
                        for kc in range(nk):
                            nc.tensor.matmul(
                                ps[:], lhsT=hT_t[kc][:], rhs=wsb[f"{wn}_{kc}"][:],
                                start=(kc == 0), stop=False,
                            )
                        nc.tensor.matmul(
                            ps[:], lhsT=ones_row[:], rhs=wsb[bn][:],
                            start=False, stop=True,
                        )
                        sb = wpool.tile([P, 2 * DM], BF, name="p1sb", tag="p1sb")
                        if half == 0:
                            nc.vector.tensor_copy(sb[:], ps[:])
                        else:
                            nc.scalar.copy(sb[:], ps[:])
                        outs.append(sb)
                    qk, vsk = outs
                    nc.sync.dma_start(out=q_dram[rows, :], in_=qk[:, 0:DM])
                    nc.sync.dma_start(
                        out=kv_agin[layer][rows, 0:DM], in_=qk[:, DM : 2 * DM]
                    )
                    nc.sync.dma_start(
                        out=kv_agin[layer][rows, DM : 2 * DM], in_=vsk[:, 0:DM]
                    )
                    nc.sync.dma_start(out=skip_dram[rows, :], in_=vsk[:, DM : 2 * DM])

            def pass2(layer, hT_next):
                for b in range(NB):
                    rows = slice(b * P, (b + 1) * P)
                    q_t = iopool.tile([P, DM], BF, name="q_t", tag="q_t")
                    nc.sync.dma_start(out=q_t[:], in_=q_dram[rows, :])
                    sk_t = iopool.tile([P, DM], BF, name="sk_t", tag="sk_t")
                    nc.sync.dma_start(out=sk_t[:], in_=skip_dram[rows, :])
                    seg = ppool.tile([P, 4 + DM], FP, name="seg", tag="seg")
                    for t in range(K_b[b]):
                        gt = off[b] + t
                        kv_g = wpool.tile([P, 2 * DM], BF, name="kv_g", tag="kv_g")
                        nc.gpsimd.indirect_dma_start(
                            out=kv_g[:],
                            out_offset=None,
                            in_=kv_full[layer][:, :],
                            in_offset=IndirectOffsetOnAxis(
                                ap=srci_s[:, gt : gt + 1], axis=0
                            ),
                        )
                        P_t = wpool.tile([P, P], BF, name="P_t", tag="P_t")
                        nc.vector.tensor_tensor(
                            out=P_t[:],
                            in0=dstl_s[:, gt : gt + 1].to_broadcast([P, P]),
                            in1=iota_f[:],
                            op=ALU.is_equal,
                        )
                        ptp = ppool.tile([P, P], BF, name="ptp", tag="ptp")
                        nc.tensor.transpose(ptp[:], P_t[:], identity[:])
                        PT_s = wpool.tile([P, P], BF, name="PT_s", tag="PT_s")
                        nc.scalar.copy(PT_s[:], ptp[:])
                        qe = ppool.tile([P, DM], FP, name="qe", tag="qe")
                        nc.tensor.matmul(
                            qe[:], lhsT=PT_s[:], rhs=q_t[:], start=True, stop=True
                        )
                        payload = wpool.tile([P, 4 + DM], BF, name="payload", tag="payload")
                        alpha4 = wpool.tile([P, 4], FP, name="alpha4", tag="alpha4")
                        prod = wpool.tile([P, DM], FP, name="prod", tag="prod")
                        nc.vector.tensor_mul(prod[:], qe[:], kv_g[:, 0:DM])
                        nc.vector.tensor_reduce(
                            out=alpha4[:],
                            in_=prod[:].rearrange("p (h c) -> p h c", c=HID),
                            axis=mybir.AxisListType.X,
                            op=ALU.add,
                        )
                        ex4 = wpool.tile([P, 4], FP, name="ex4", tag="ex4")
                        nc.scalar.activation(
                            ex4[:], alpha4[:], AF.Exp, scale=1.0 / math.sqrt(HID)
                        )
                        nc.scalar.copy(payload[:, 0:4], ex4[:])
                        for h in range(H):
                            nc.vector.tensor_scalar(
                                out=payload[:, 4 + h * HID : 4 + (h + 1) * HID],
                                in0=kv_g[:, DM + h * HID : DM + (h + 1) * HID],
                                scalar1=ex4[:, h : h + 1],
                                scalar2=None,
                                op0=ALU.mult,
                            )
                        nc.tensor.matmul(
                            seg[:], lhsT=P_t[:], rhs=payload[:],
                            start=(t == 0), stop=(t == K_b[b] - 1),
                        )
                    # epilogue
                    rec4 = wpool.tile([P, 4], FP, name="rec4", tag="rec4")
                    nc.vector.tensor_scalar(
                        out=rec4[:], in0=seg[:, 0:4], scalar1=1e-20, scalar2=None,
                        op0=ALU.max,
                    )
                    nc.vector.reciprocal(rec4[:], rec4[:])
                    hb = wpool.tile([P, DM], FP, name="hb", tag="hb")
                    for h in range(H):
                        nc.vector.tensor_scalar(
                            out=hb[:, h * HID : (h + 1) * HID],
                            in0=seg[:, 4 + h * HID : 4 + (h + 1) * HID],
                            scalar1=rec4[:, h : h + 1],
                            scalar2=None,
                            op0=ALU.mult,
                        )
                    nc.vector.tensor_add(hb[:], hb[:], sk_t[:])
                    hr = wpool.tile([P, DM], BF, name="hr", tag="hr")
                    nc.scalar.activation(hr[:], hb[:], AF.Relu)
                    if hT_next is not None:
                        if USE_TDMA:
                            nc.sync.dma_start(out=hT_next[rows, :], in_=hr[:])
                        else:
                            for half in range(2):
                                tp = ppool.tile([P, P], BF, name="ptp", tag="ptp")
                                nc.tensor.transpose(
                                    tp[:], hr[:, half * P : (half + 1) * P],
                                    identity[:],
                                )
                                ts = wpool.tile([P, P], BF, name="ts", tag="ts")
                                nc.vector.tensor_copy(ts[:], tp[:])
                                nc.sync.dma_start(
                                    out=hT_next[half * P : (half + 1) * P, rows],
                                    in_=ts[:],
                                )
                    else:
                        B_t = wpool.tile([P, G], BF, name="B_t", tag="B_t")
                        nc.vector.tensor_tensor(
                            out=B_t[:],
                            in0=batch_s[:, b : b + 1].to_broadcast([P, G]),
                            in1=iota_f[:, 0:G],
                            op=ALU.is_equal,
                        )
                        for half, acc in ((0, pool_acc0), (1, pool_acc1)):
                            pp = ppool.tile([P, G], FP, name="ptp", tag="ptp")
                            nc.tensor.matmul(
                                pp[:], lhsT=hr[:, half * P : (half + 1) * P],
                                rhs=B_t[:], start=True, stop=True,
                            )
                            nc.vector.tensor_add(acc[:], acc[:], pp[:])

            def ag_kv(layer):
                nc.gpsimd.collective_compute(
                    "AllGather",
                    ALU.bypass,
                    replica_groups=rg,
                    ins=[kv_agin[layer].opt()],
                    outs=[kv_full[layer].opt()],
                )

            pass1(0, xT_in)
            ag_kv(0)
            pass2(0, h_a)
            pass1(1, h_a)
            ag_kv(1)
            pass2(1, h_b)
            pass1(2, h_b)
            ag_kv(2)
            pass2(2, None)

            # ---------------- head
            nc.sync.dma_start(out=pool_in[0:P, :], in_=pool_acc0[:])
            nc.sync.dma_start(out=pool_in[P : 2 * P, :], in_=pool_acc1[:])
            nc.gpsimd.collective_compute(
                "AllReduce",
                ALU.add,
                replica_groups=rg,
                ins=[pool_in.opt()],
                outs=[pool_red.opt()],
            )
            xp = []
            for half in range(2):
                t = iopool.tile([P, G], FP, name="poolr", tag="poolr")
                nc.sync.dma_start(out=t[:], in_=pool_red[half * P : (half + 1) * P, :])
                m = cpool.tile([P, G], FP, name=f"xp{half}")
                nc.vector.tensor_mul(m[:], t[:], invb[:])
                xp.append(m)
            # global mlp branch (feature-major [GH, G])
            g_cur = wsb["gT"]
            for i in range(3):
                gp = ppool.tile([GH, G], FP, name="p1", tag="p1")
                nc.tensor.matmul(
                    gp[:], lhsT=wsb[f"wg{i}"][:], rhs=g_cur[:], start=True, stop=True
                )
                gs = cpool.tile([GH, G], FP, name=f"g_{i}")
                nc.scalar.activation(
                    gs[:], gp[:], AF.Relu, bias=wsb[f"bg{i}"][:, 0:1]
                )
                g_cur = gs
            # regressor
            r0p = ppool.tile([RH, G], FP, name="p1", tag="p1")
            nc.tensor.matmul(r0p[:], lhsT=wsb["w0T_0"][:], rhs=xp[0][:], start=True, stop=False)
            nc.tensor.matmul(r0p[:], lhsT=wsb["w0T_1"][:], rhs=xp[1][:], start=False, stop=False)
            nc.tensor.matmul(r0p[:], lhsT=wsb["w0T_2"][:], rhs=g_cur[:], start=False, stop=True)
            r0 = cpool.tile([RH, G], FP, name="r0")
            nc.scalar.activation(r0[:], r0p[:], AF.Relu, bias=wsb["b0"][:, 0:1])
            r1p = ppool.tile([RH, G], FP, name="p1", tag="p1")
            nc.tensor.matmul(r1p[:], lhsT=wsb["w1T"][:], rhs=r0[:], start=True, stop=True)
            r1 = cpool.tile([RH, G], FP, name="r1")
            nc.scalar.activation(r1[:], r1p[:], AF.Relu, bias=wsb["b1"][:, 0:1])
            r2p = ppool.tile([1, G], FP, name="p1", tag="p1")
            nc.tensor.matmul(r2p[:], lhsT=wsb["w2T"][:], rhs=r1[:], start=True, stop=True)
            outs = cpool.tile([1, G], FP, name="outs")
            nc.scalar.activation(outs[:], r2p[:], AF.Identity, bias=wsb["b2"][:, 0:1])
            nc.sync.dma_start(out=out_dram[:, :], in_=outs[:])

    nc.compile()
    return nc


def _in_maps(meta, percore, weights):
    maps = []
    for c in range(NCORES):
        m = dict(
            xT=percore["xT"][c],
            src_idx=percore["src_idx"][c],
            dst_loc=percore["dst_loc"][c],
            batchf=percore["batchf"][c],
        )
        m.update(weights)
        maps.append(m)
    return maps


LAST_RESULT = None


def kernel(x, global_features, params, edge_index, batch):
    global LAST_RESULT
    from concourse.bass_utils import run_bass_kernel_spmd

    meta, percore, weights = _prepare(x, global_features, params, edge_index, batch)
    nc = _build(meta)
    maps = _in_maps(meta, percore, weights)
    res = run_bass_kernel_spmd(nc, maps, core_ids=list(range(NCORES)))
    LAST_RESULT = res
    return np.asarray(res.results[0]["out"], dtype=np.float32).reshape(-1)


# revision 10
# speedup vs baseline: 1.7875x; 1.2325x over previous
"""CircuitGNN (3x TransformerConv + pool + MLP head) on 8 trn2 NeuronCores.

Strategy (graph/node-parallel with dst-side edge ownership):
  - Nodes are split into 8 contiguous ranges (padded to a multiple of 128).
  - Each layer: every core computes q/k/v/skip projections for its own
    nodes; k|v are packed into one [Np,512] table and AllGather'd so every
    core holds the full [8*Np,512] table; edges (sorted by destination,
    owned by the destination's core) gather k|v rows by source id via
    indirect DMA; attention + segment softmax + aggregation are computed
    with one-hot matrices on the tensor engine (segment-sum == matmul with
    a one-hot selection matrix).  exp() needs no max subtraction: logits
    are O(0.03) for this model family, and softmax is shift-invariant.
  - Mean-pool runs as a one-hot matmul accumulated over node blocks, an
    AllReduce combines the per-core partials, and the tiny global-MLP +
    regressor head runs replicated on every core.
"""

import math

import numpy as np

import concourse.bass as bass
import concourse.bacc as bacc
import concourse.mybir as mybir
import concourse.tile as tile
from concourse.bass import IndirectOffsetOnAxis
from concourse.masks import make_identity

BF16_NP = mybir.dt.np(mybir.dt.bfloat16)

P = 128
NCORES = 8
H = 4
HID = 64
DM = H * HID  # 256
FP = mybir.dt.float32
BF = mybir.dt.bfloat16
I32 = mybir.dt.int32
AF = mybir.ActivationFunctionType
ALU = mybir.AluOpType


# ---------------------------------------------------------------- host prep
def _prepare(x, global_features, params, edge_index, batch):
    N, D_IN = x.shape
    E = edge_index.shape[1]
    G = global_features.shape[0]

    src = np.asarray(edge_index[0], dtype=np.int64)
    dst = np.asarray(edge_index[1], dtype=np.int64)
    batch = np.asarray(batch, dtype=np.int64)

    n_per = math.ceil(N / NCORES)
    Np = math.ceil(n_per / P) * P
    NB = Np // P

    # global padded row id for a node
    core_of = src // n_per
    src_pad = core_of * Np + (src - core_of * n_per)

    order = np.argsort(dst, kind="stable")
    dst_s = dst[order]
    srcp_s = src_pad[order]

    # per (core, block) edge counts
    counts = np.zeros((NCORES, NB), dtype=np.int64)
    core_lims = np.searchsorted(dst_s, [c * n_per for c in range(NCORES + 1)])
    for c in range(NCORES):
        lo, hi = core_lims[c], core_lims[c + 1]
        dl = dst_s[lo:hi] - c * n_per
        blk_lims = np.searchsorted(dl, [b * P for b in range(NB + 1)])
        counts[c] = np.diff(blk_lims)

    K_b = np.maximum(1, np.ceil(counts.max(axis=0) / P).astype(np.int64))
    off = np.concatenate([[0], np.cumsum(K_b)])
    T_total = int(off[-1])

    src_idx = np.zeros((NCORES, T_total * P), dtype=np.int32)
    dst_loc = np.full((NCORES, T_total * P), -1.0, dtype=np.float32)
    for c in range(NCORES):
        lo, hi = core_lims[c], core_lims[c + 1]
        dl = dst_s[lo:hi] - c * n_per
        sp = srcp_s[lo:hi]
        blk_lims = np.searchsorted(dl, [b * P for b in range(NB + 1)])
        for b in range(NB):
            blo, bhi = blk_lims[b], blk_lims[b + 1]
            cnt = bhi - blo
            pos = off[b] * P + np.arange(cnt)
            src_idx[c, pos] = sp[blo:bhi]
            dst_loc[c, pos] = (dl[blo:bhi] - b * P).astype(np.float32)
    # [128, T_total] layout: col = tile, row = slot within tile
    src_idx = np.ascontiguousarray(
        src_idx.reshape(NCORES, T_total, P).transpose(0, 2, 1)
    )
    dst_loc = np.ascontiguousarray(
        dst_loc.reshape(NCORES, T_total, P).transpose(0, 2, 1)
    )
    # one-hot selection matrices per edge tile (and transposes), bf16
    P_host = np.zeros((NCORES, P, T_total * P), dtype=BF16_NP)
    PT_host = np.zeros((NCORES, P, T_total * P), dtype=BF16_NP)
    for c in range(NCORES):
        dl = dst_loc[c]  # [P, T_total]
        e_slot, t_idx = np.nonzero(dl >= 0)
        d_local = dl[e_slot, t_idx].astype(np.int64)
        P_host[c, e_slot, t_idx * P + d_local] = 1
        PT_host[c, d_local, t_idx * P + e_slot] = 1

    # batch (graph id) per local node, [128, NB]; -1 on padded rows
    batchf = np.full((NCORES, Np), -1.0, dtype=np.float32)
    for c in range(NCORES):
        n_c = min(n_per, N - c * n_per)
        if n_c > 0:
            batchf[c, :n_c] = batch[c * n_per : c * n_per + n_c].astype(np.float32)
    batchf = np.ascontiguousarray(batchf.reshape(NCORES, NB, P).transpose(0, 2, 1))

    # node features, feature-major per core
    xT = np.zeros((NCORES, D_IN, Np), dtype=BF16_NP)
    for c in range(NCORES):
        n_c = min(n_per, N - c * n_per)
        if n_c > 0:
            xT[c, :, :n_c] = x[c * n_per : c * n_per + n_c].T.astype(BF16_NP)

    f32 = lambda a: np.ascontiguousarray(np.asarray(a, dtype=np.float32))
    bf16 = lambda a: np.ascontiguousarray(np.asarray(a, dtype=np.float32).astype(BF16_NP))
    weights = {}
    dins = []
    for i in range(3):
        p = params[f"conv{i}"]
        din = p["Wq"].shape[1]
        dins.append(din)
        weights[f"wqk{i}"] = bf16(np.concatenate([p["Wq"].T, p["Wk"].T], axis=1))
        weights[f"wvsk{i}"] = bf16(np.concatenate([p["Wv"].T, p["Wskip"].T], axis=1))
        weights[f"bqk{i}"] = bf16(np.concatenate([p["bq"], p["bk"]]).reshape(1, 2 * DM))
        weights[f"bvsk{i}"] = bf16(
            np.concatenate([p["bv"], p["bskip"]]).reshape(1, 2 * DM)
        )
    for i, (W, b) in enumerate(params["gmlp"]):
        weights[f"wg{i}"] = f32(W.T)  # [K, GH]
        weights[f"bg{i}"] = f32(np.asarray(b).reshape(-1, 1))  # [GH, 1]
    (W0, b0), (W1, b1), (W2, b2) = params["reg"]
    weights["w0T"] = f32(W0.T)  # [DM+GH, RH]
    weights["b0"] = f32(np.asarray(b0).reshape(-1, 1))
    weights["w1T"] = f32(W1.T)
    weights["b1"] = f32(np.asarray(b1).reshape(-1, 1))
    weights["w2T"] = f32(W2.T)  # [RH, 1]
    weights["b2"] = f32(np.asarray(b2).reshape(1, 1))
    weights["gT"] = f32(np.asarray(global_features).T)  # [GIN, G]
    cnts = np.bincount(batch, minlength=G).astype(np.float32)
    weights["invcnt"] = f32((1.0 / np.maximum(cnts, 1.0)).reshape(1, G))

    meta = dict(
        N=N, E=E, G=G, D_IN=D_IN, GH=params["gmlp"][0][0].shape[0],
        RH=W0.shape[0], GIN=global_features.shape[1],
        n_per=n_per, Np=Np, NB=NB, K_b=[int(k) for k in K_b],
        off=[int(o) for o in off], T_total=T_total, dins=dins,
    )
    percore = dict(src_idx=src_idx, batchf=batchf, xT=xT,
                   P_host=P_host, PT_host=PT_host)
    return meta, percore, weights


# ---------------------------------------------------------------- program
USE_TDMA = False  # transpose-DMA loads for h (vs PE-transpose in epilogue)


def _build(meta):
    Np, NB = meta["Np"], meta["NB"]
    K_b, off, T_total = meta["K_b"], meta["off"], meta["T_total"]
    G, GH, RH, GIN, D_IN = meta["G"], meta["GH"], meta["RH"], meta["GIN"], meta["D_IN"]
    dins = meta["dins"]

    nc = bacc.Bacc(
        "TRN2", target_bir_lowering=False, debug=False, num_devices=NCORES
    )
    rg = [list(range(NCORES))]

    # -------- I/O
    xT_in = nc.dram_tensor("xT", [D_IN, Np], BF, kind="ExternalInput")
    srci_in = nc.dram_tensor("src_idx", [P, T_total], I32, kind="ExternalInput")
    pb_in = nc.dram_tensor("P_host", [P, T_total * P], BF, kind="ExternalInput")
    ptb_in = nc.dram_tensor("PT_host", [P, T_total * P], BF, kind="ExternalInput")
    batch_in = nc.dram_tensor("batchf", [P, NB], FP, kind="ExternalInput")
    win = {}
    for i in range(3):
        din = dins[i]
        win[f"wqk{i}"] = nc.dram_tensor(f"wqk{i}", [din, 2 * DM], BF, kind="ExternalInput")
        win[f"wvsk{i}"] = nc.dram_tensor(f"wvsk{i}", [din, 2 * DM], BF, kind="ExternalInput")
        win[f"bqk{i}"] = nc.dram_tensor(f"bqk{i}", [1, 2 * DM], BF, kind="ExternalInput")
        win[f"bvsk{i}"] = nc.dram_tensor(f"bvsk{i}", [1, 2 * DM], BF, kind="ExternalInput")
    gk = [GIN, GH, GH]
    for i in range(3):
        win[f"wg{i}"] = nc.dram_tensor(f"wg{i}", [gk[i], GH], FP, kind="ExternalInput")
        win[f"bg{i}"] = nc.dram_tensor(f"bg{i}", [GH, 1], FP, kind="ExternalInput")
    win["w0T"] = nc.dram_tensor("w0T", [DM + GH, RH], FP, kind="ExternalInput")
    win["b0"] = nc.dram_tensor("b0", [RH, 1], FP, kind="ExternalInput")
    win["w1T"] = nc.dram_tensor("w1T", [RH, RH], FP, kind="ExternalInput")
    win["b1"] = nc.dram_tensor("b1", [RH, 1], FP, kind="ExternalInput")
    win["w2T"] = nc.dram_tensor("w2T", [RH, 1], FP, kind="ExternalInput")
    win["b2"] = nc.dram_tensor("b2", [1, 1], FP, kind="ExternalInput")
    win["gT"] = nc.dram_tensor("gT", [GIN, G], FP, kind="ExternalInput")
    win["invcnt"] = nc.dram_tensor("invcnt", [1, G], FP, kind="ExternalInput")
    out_dram = nc.dram_tensor("out", [1, G], FP, kind="ExternalOutput")

    with tile.TileContext(nc) as tc:
        with (
            tc.tile_pool(name="const", bufs=1) as cpool,
            tc.tile_pool(name="dram", bufs=1, space="DRAM") as dpool,
            tc.tile_pool(name="work", bufs=4) as wpool,
            tc.tile_pool(name="io", bufs=3) as iopool,
            tc.tile_pool(name="psum", bufs=2, space="PSUM") as ppool,
        ):
            # ---- persistent DRAM scratch
            if USE_TDMA:
                h_a = dpool.tile([Np, DM], BF, name="h_a")
                h_b = dpool.tile([Np, DM], BF, name="h_b")
            else:
                h_a = dpool.tile([DM, Np], BF, name="h_a")
                h_b = dpool.tile([DM, Np], BF, name="h_b")
            q_dram = dpool.tile([Np, DM], BF, name="q_dram")
            skip_dram = dpool.tile([Np, DM], BF, name="skip_dram")
            kv_agin = [
                dpool.tile([Np, 2 * DM], BF, name=f"kv_agin{i}") for i in range(3)
            ]
            kv_full = [
                dpool.tile(
                    [NCORES * Np, 2 * DM], BF, name=f"kv_full{i}", addr_space="Shared"
                )
                for i in range(3)
            ]
            pool_in = dpool.tile([DM, G], FP, name="pool_in")
            pool_red = dpool.tile([DM, G], FP, name="pool_red", addr_space="Shared")

            # ---- constants / persistent SBUF
            identity = cpool.tile([P, P], BF, name="identity")
            make_identity(nc, identity[:])
            iota_f = cpool.tile([P, P], FP, name="iota_f")
            nc.gpsimd.iota(
                iota_f[:], [[1, P]], channel_multiplier=0,
                allow_small_or_imprecise_dtypes=True,
            )
            ones_row = cpool.tile([1, P], BF, name="ones_row")
            nc.gpsimd.memset(ones_row[:], 1.0)

            srci_s = cpool.tile([P, T_total], I32, name="srci_s")
            nc.sync.dma_start(out=srci_s[:], in_=srci_in[:, :])
            batch_s = cpool.tile([P, NB], FP, name="batch_s")
            nc.sync.dma_start(out=batch_s[:], in_=batch_in[:, :])

            wsb = {}
            for i in range(3):
                din = dins[i]
                nk = math.ceil(din / P)
                for nm in (f"wqk{i}", f"wvsk{i}"):
                    for kc in range(nk):
                        klo, khi = kc * P, min((kc + 1) * P, din)
                        t = cpool.tile([khi - klo, 2 * DM], BF, name=f"{nm}_{kc}")
                        nc.sync.dma_start(out=t[:], in_=win[nm][klo:khi, :])
                        wsb[f"{nm}_{kc}"] = t
                for nm in (f"bqk{i}", f"bvsk{i}"):
                    t = cpool.tile([1, 2 * DM], BF, name=f"{nm}_s")
                    nc.sync.dma_start(out=t[:], in_=win[nm][:, :])
                    wsb[nm] = t
            for nm, shp in [
                ("wg0", [gk[0], GH]), ("wg1", [gk[1], GH]), ("wg2", [gk[2], GH]),
                ("bg0", [GH, 1]), ("bg1", [GH, 1]), ("bg2", [GH, 1]),
                ("b0", [RH, 1]), ("b1", [RH, 1]), ("w2T", [RH, 1]), ("b2", [1, 1]),
                ("w1T", [RH, RH]), ("gT", [GIN, G]),
            ]:
                t = cpool.tile(shp, FP, name=f"{nm}_s")
                nc.sync.dma_start(out=t[:], in_=win[nm][:, :])
                wsb[nm] = t
            for kc in range(3):  # w0T chunks: [128,RH],[128,RH],[GH,RH]
                klo = kc * P
                khi = min(klo + P, DM + GH)
                t = cpool.tile([khi - klo, RH], FP, name=f"w0T_{kc}")
                nc.sync.dma_start(out=t[:], in_=win["w0T"][klo:khi, :])
                wsb[f"w0T_{kc}"] = t
            inv1 = cpool.tile([1, G], FP, name="inv1")
            nc.sync.dma_start(out=inv1[:], in_=win["invcnt"][:, :])
            invb = cpool.tile([P, G], FP, name="invb")
            nc.gpsimd.partition_broadcast(invb[:], inv1[:])

            pool_acc0 = cpool.tile([P, G], FP, name="pool_acc0")
            pool_acc1 = cpool.tile([P, G], FP, name="pool_acc1")
            nc.vector.memset(pool_acc0[:], 0.0)
            nc.vector.memset(pool_acc1[:], 0.0)

            # ---------------- per-layer passes
            def pass1(layer, h_src):
                din = dins[layer]
                nk = math.ceil(din / P)
                for nt in range(NB):
                    rows = slice(nt * P, (nt + 1) * P)
                    hT_t = []
                    for kc in range(nk):
                        klo, khi = kc * P, min((kc + 1) * P, din)
                        t = wpool.tile([khi - klo, P], BF, name="hT_t", tag="hT_t")
                        if layer == 0 or not USE_TDMA:
                            nc.sync.dma_start(out=t[:], in_=h_src[klo:khi, rows])
                        else:
                            nc.sync.dma_start(
                                out=t[:], in_=h_src[rows, klo:khi], transpose=True
                            )
                        hT_t.append(t)
                    outs = []
                    for half, wn, bn in (
                        (0, f"wqk{layer}", f"bqk{layer}"),
                        (1, f"wvsk{layer}", f"bvsk{layer}"),
                    ):
                        ps = ppool.tile([P, 2 * DM], FP, name="p1", tag="p1")
                        for kc in range(nk):
                            nc.tensor.matmul(
                                ps[:], lhsT=hT_t[kc][:], rhs=wsb[f"{wn}_{kc}"][:],
                                start=(kc == 0), stop=False,
                            )
                        nc.tensor.matmul(
                            ps[:], lhsT=ones_row[:], rhs=wsb[bn][:],
                            start=False, stop=True,
                        )
                        sb = wpool.tile([P, 2 * DM], BF, name="p1sb", tag="p1sb")
                        if half == 0:
                            nc.vector.tensor_copy(sb[:], ps[:])
                        else:
                            nc.scalar.copy(sb[:], ps[:])
                        outs.append(sb)
                    qk, vsk = outs
                    nc.sync.dma_start(out=q_dram[rows, :], in_=qk[:, 0:DM])
                    nc.sync.dma_start(
                        out=kv_agin[layer][rows, 0:DM], in_=qk[:, DM : 2 * DM]
                    )
                    nc.sync.dma_start(
                        out=kv_agin[layer][rows, DM : 2 * DM], in_=vsk[:, 0:DM]
                    )
                    nc.sync.dma_start(out=skip_dram[rows, :], in_=vsk[:, DM : 2 * DM])

            def pass2(layer, hT_next):
                for b in range(NB):
                    rows = slice(b * P, (b + 1) * P)
                    q_t = iopool.tile([P, DM], BF, name="q_t", tag="q_t")
                    nc.sync.dma_start(out=q_t[:], in_=q_dram[rows, :])
                    sk_t = iopool.tile([P, DM], BF, name="sk_t", tag="sk_t")
                    nc.sync.dma_start(out=sk_t[:], in_=skip_dram[rows, :])
                    Pb = iopool.tile([P, K_b[b] * P], BF, name="Pb", tag="Pb")
                    nc.sync.dma_start(
                        out=Pb[:], in_=pb_in[:, off[b] * P : (off[b] + K_b[b]) * P]
                    )
                    PTb = iopool.tile([P, K_b[b] * P], BF, name="PTb", tag="PTb")
                    nc.sync.dma_start(
                        out=PTb[:], in_=ptb_in[:, off[b] * P : (off[b] + K_b[b]) * P]
                    )
                    seg = ppool.tile([P, 4 + DM], FP, name="seg", tag="seg")
                    for t in range(K_b[b]):
                        gt = off[b] + t
                        kv_g = wpool.tile([P, 2 * DM], BF, name="kv_g", tag="kv_g")
                        nc.gpsimd.indirect_dma_start(
                            out=kv_g[:],
                            out_offset=None,
                            in_=kv_full[layer][:, :],
                            in_offset=IndirectOffsetOnAxis(
                                ap=srci_s[:, gt : gt + 1], axis=0
                            ),
                        )
                        qe = ppool.tile([P, DM], FP, name="qe", tag="qe")
                        nc.tensor.matmul(
                            qe[:], lhsT=PTb[:, t * P : (t + 1) * P], rhs=q_t[:],
                            start=True, stop=True,
                        )
                        payload = wpool.tile([P, 4 + DM], BF, name="payload", tag="payload")
                        alpha4 = wpool.tile([P, 4], FP, name="alpha4", tag="alpha4")
                        prod = wpool.tile([P, DM], FP, name="prod", tag="prod")
                        nc.vector.tensor_mul(prod[:], qe[:], kv_g[:, 0:DM])
                        nc.vector.tensor_reduce(
                            out=alpha4[:],
                            in_=prod[:].rearrange("p (h c) -> p h c", c=HID),
                            axis=mybir.AxisListType.X,
                            op=ALU.add,
                        )
                        nc.scalar.activation(
                            payload[:, 0:4], alpha4[:], AF.Exp,
                            scale=1.0 / math.sqrt(HID),
                        )
                        exb = payload[:, 0:4]
                        ex_exp = bass.AP(
                            exb.tensor, exb.offset, [exb.ap[0], [1, H], [0, HID]]
                        )
                        nc.vector.tensor_tensor(
                            out=payload[:, 4 : 4 + DM].rearrange(
                                "p (h c) -> p h c", c=HID
                            ),
                            in0=kv_g[:, DM : 2 * DM].rearrange(
                                "p (h c) -> p h c", c=HID
                            ),
                            in1=ex_exp,
                            op=ALU.mult,
                        )
                        nc.tensor.matmul(
                            seg[:], lhsT=Pb[:, t * P : (t + 1) * P], rhs=payload[:],
                            start=(t == 0), stop=(t == K_b[b] - 1),
                        )
                    # epilogue
                    rec4 = wpool.tile([P, 4], FP, name="rec4", tag="rec4")
                    nc.vector.tensor_scalar(
                        out=rec4[:], in0=seg[:, 0:4], scalar1=1e-20, scalar2=None,
                        op0=ALU.max,
                    )
                    nc.vector.reciprocal(rec4[:], rec4[:])
                    hb = wpool.tile([P, DM], FP, name="hb", tag="hb")
                    r4 = rec4[:, 0:4]
                    r4_exp = bass.AP(
                        r4.tensor, r4.offset, [r4.ap[0], [1, H], [0, HID]]
                    )
                    nc.vector.tensor_tensor(
                        out=hb[:].rearrange("p (h c) -> p h c", c=HID),
                        in0=seg[:, 4 : 4 + DM].rearrange("p (h c) -> p h c", c=HID),
                        in1=r4_exp,
                        op=ALU.mult,
                    )
                    nc.vector.tensor_add(hb[:], hb[:], sk_t[:])
                    hr = wpool.tile([P, DM], BF, name="hr", tag="hr")
                    nc.scalar.activation(hr[:], hb[:], AF.Relu)
                    if hT_next is not None:
                        if USE_TDMA:
                            nc.sync.dma_start(out=hT_next[rows, :], in_=hr[:])
                        else:
                            for half in range(2):
                                tp = ppool.tile([P, P], BF, name="ptp", tag="ptp")
                                nc.tensor.transpose(
                                    tp[:], hr[:, half * P : (half + 1) * P],
                                    identity[:],
                                )
                                ts = wpool.tile([P, P], BF, name="ts", tag="ts")
                                nc.vector.tensor_copy(ts[:], tp[:])
                                nc.sync.dma_start(
                                    out=hT_next[half * P : (half + 1) * P, rows],
                                    in_=ts[:],
                                )
                    else:
                        B_t = wpool.tile([P, G], BF, name="B_t", tag="B_t")
                        nc.vector.tensor_tensor(
                            out=B_t[:],
                            in0=batch_s[:, b : b + 1].to_broadcast([P, G]),
                            in1=iota_f[:, 0:G],
                            op=ALU.is_equal,
                        )
                        for half, acc in ((0, pool_acc0), (1, pool_acc1)):
                            pp = ppool.tile([P, G], FP, name="ptp", tag="ptp")
                            nc.tensor.matmul(
                                pp[:], lhsT=hr[:, half * P : (half + 1) * P],
                                rhs=B_t[:], start=True, stop=True,
                            )
                            nc.vector.tensor_add(acc[:], acc[:], pp[:])

            def ag_kv(layer):
                nc.gpsimd.collective_compute(
                    "AllGather",
                    ALU.bypass,
                    replica_groups=rg,
                    ins=[kv_agin[layer].opt()],
                    outs=[kv_full[layer].opt()],
                )

            pass1(0, xT_in)
            ag_kv(0)
            pass2(0, h_a)
            pass1(1, h_a)
            ag_kv(1)
            pass2(1, h_b)
            pass1(2, h_b)
            ag_kv(2)
            pass2(2, None)

            # ---------------- head
            nc.sync.dma_start(out=pool_in[0:P, :], in_=pool_acc0[:])
            nc.sync.dma_start(out=pool_in[P : 2 * P, :], in_=pool_acc1[:])
            nc.gpsimd.collective_compute(
                "AllReduce",
                ALU.add,
                replica_groups=rg,
                ins=[pool_in.opt()],
                outs=[pool_red.opt()],
            )
            xp = []
            for half in range(2):
                t = iopool.tile([P, G], FP, name="poolr", tag="poolr")
                nc.sync.dma_start(out=t[:], in_=pool_red[half * P : (half + 1) * P, :])
                m = cpool.tile([P, G], FP, name=f"xp{half}")
                nc.vector.tensor_mul(m[:], t[:], invb[:])
                xp.append(m)
            # global mlp branch (feature-major [GH, G])
            g_cur = wsb["gT"]
            for i in range(3):
                gp = ppool.tile([GH, G], FP, name="p1", tag="p1")
                nc.tensor.matmul(
                    gp[:], lhsT=wsb[f"wg{i}"][:], rhs=g_cur[:], start=True, stop=True
                )
                gs = cpool.tile([GH, G], FP, name=f"g_{i}")
                nc.scalar.activation(
                    gs[:], gp[:], AF.Relu, bias=wsb[f"bg{i}"][:, 0:1]
                )
                g_cur = gs
            # regressor
            r0p = ppool.tile([RH, G], FP, name="p1", tag="p1")
            nc.tensor.matmul(r0p[:], lhsT=wsb["w0T_0"][:], rhs=xp[0][:], start=True, stop=False)
            nc.tensor.matmul(r0p[:], lhsT=wsb["w0T_1"][:], rhs=xp[1][:], start=False, stop=False)
            nc.tensor.matmul(r0p[:], lhsT=wsb["w0T_2"][:], rhs=g_cur[:], start=False, stop=True)
            r0 = cpool.tile([RH, G], FP, name="r0")
            nc.scalar.activation(r0[:], r0p[:], AF.Relu, bias=wsb["b0"][:, 0:1])
            r1p = ppool.tile([RH, G], FP, name="p1", tag="p1")
            nc.tensor.matmul(r1p[:], lhsT=wsb["w1T"][:], rhs=r0[:], start=True, stop=True)
            r1 = cpool.tile([RH, G], FP, name="r1")
            nc.scalar.activation(r1[:], r1p[:], AF.Relu, bias=wsb["b1"][:, 0:1])
            r2p = ppool.tile([1, G], FP, name="p1", tag="p1")
            nc.tensor.matmul(r2p[:], lhsT=wsb["w2T"][:], rhs=r1[:], start=True, stop=True)
            outs = cpool.tile([1, G], FP, name="outs")
            nc.scalar.activation(outs[:], r2p[:], AF.Identity, bias=wsb["b2"][:, 0:1])
            nc.sync.dma_start(out=out_dram[:, :], in_=outs[:])

    nc.compile()
    return nc


def _in_maps(meta, percore, weights):
    maps = []
    for c in range(NCORES):
        m = dict(
            xT=percore["xT"][c],
            src_idx=percore["src_idx"][c],
            batchf=percore["batchf"][c],
            P_host=percore["P_host"][c],
            PT_host=percore["PT_host"][c],
        )
        m.update(weights)
        maps.append(m)
    return maps


LAST_RESULT = None


def kernel(x, global_features, params, edge_index, batch):
    global LAST_RESULT
    from concourse.bass_utils import run_bass_kernel_spmd

    meta, percore, weights = _prepare(x, global_features, params, edge_index, batch)
    nc = _build(meta)
    maps = _in_maps(meta, percore, weights)
    res = run_bass_kernel_spmd(nc, maps, core_ids=list(range(NCORES)))
    LAST_RESULT = res
    return np.asarray(res.results[0]["out"], dtype=np.float32).reshape(-1)
